# revision 32
# baseline (speedup 1.0000x reference)
"""Trainium2 Bass kernel for nn_MoDBlock (mixture-of-depths transformer block).

Sharding: data-parallel over batch B=8 across the 8 NeuronCores (one batch row
per core; routing/gather/scatter are per-row independent). Everything runs
on-device per core:

  logits  = x @ Wr                      (DVE fused mul+reduce, 32 tiles)
  thr     = exact 512th-largest logit via 5-stage counting search:
            128 candidate thresholds/stage on DVE (is_ge + accum count),
            flag-sum via PE matmul, interval refined x128 per stage down to
            < 1 ULP -> threshold t with #(logits >= t) == 512 exactly.
  sel, w  = ascending-index compaction  (gpsimd sparse_gather on masked iota /
                                         shifted logits)
  tok     = dma_gather(x, sel)          (512 rows of 4KB)
  block   = pre-LN attention + SwiGLU MLP; fp8e4 DoubleRow matmuls (2x PE
            throughput) for QKV/W1/W3/W2 with f32 accumulation; weights
            pre-scaled x64 and LN outputs x16 on-chip (fp8 denormal
            avoidance), rescaled 2^-10 at PSUM eviction; bf16 attention;
            f32 LN/softmax statistics; softmax without max-subtraction
            (|scores/8| < 3 at this operator's scale), causal mask applied
            multiplicatively only on the diagonal 128x128 block.
  out     = copy of x (written from the stage-1 tiles; x is read once),
            then dma_scatter_add(out, proc * w, sel)

Host-side preprocessing: weights cast to fp8e4 (x64, clipped +-240) / Wo to
bf16, LN gains folded into Wqkv/W1/W3 rows.
"""

import os
import numpy as np
import ml_dtypes

import concourse.bass as bass
import concourse.mybir as mybir
import concourse.tile as tile
from concourse import bacc, masks
from concourse.bass_utils import run_bass_kernel_spmd

F32 = mybir.dt.float32
BF16 = mybir.dt.bfloat16
FP8 = mybir.dt.float8e4
I16 = mybir.dt.int16
I32 = mybir.dt.int32
U32 = mybir.dt.uint32
AF = mybir.ActivationFunctionType
ALU = mybir.AluOpType
DR = mybir.MatmulPerfMode.DoubleRow

B, T, C = 8, 4096, 1024
H, DH, FF = 16, 64, 4096
K = 512                      # routed tokens per batch row
EPS = 1e-5
NT = T // 128                # 32 x-tiles
NI = K // 128                # 4 reduced-seq token chunks
NCC = C // 128               # 8 feature chunks
NCP = NCC // 2               # 4 feature pair-chunks (DoubleRow)
NFC = FF // 128              # 32 ffn chunks
N_CORES = 8

WS = 64.0                    # host-side weight scale (fp8)
AS = 16.0                    # on-chip LN-output scale (fp8)
INV = 1.0 / (WS * AS)        # PSUM rescale 2^-10

KSTOP = int(os.environ.get("KSTOP", "99"))


def build(nc, tc, es):
    x_d = nc.dram_tensor("x", (T, C), F32, kind="ExternalInput").ap()
    wr_d = nc.dram_tensor("wr", (1, C), F32, kind="ExternalInput").ap()
    wqkv_d = nc.dram_tensor("wqkv", (C, 3 * C), FP8, kind="ExternalInput").ap()
    wo_d = nc.dram_tensor("wo", (C, C), BF16, kind="ExternalInput").ap()
    w1_d = nc.dram_tensor("w1", (C, FF), FP8, kind="ExternalInput").ap()
    w3_d = nc.dram_tensor("w3", (C, FF), FP8, kind="ExternalInput").ap()
    w2_d = nc.dram_tensor("w2", (FF, C), FP8, kind="ExternalInput").ap()
    out_d = nc.dram_tensor("out", (T, C), F32, kind="ExternalOutput").ap()

    const = es.enter_context(tc.tile_pool(name="const", bufs=1))
    ident = const.tile([128, 128], BF16)
    masks.make_identity(nc, ident[:])
    ones_col = const.tile([128, 1], BF16)       # all-ones column
    nc.vector.memset(ones_col[:], 1.0)
    ones_row = const.tile([1, 128], BF16)       # all-ones row on partition 0
    nc.vector.memset(ones_row[:], 1.0)
    # diagonal causal mask: cmask_d[p, q] = 1.0 if q >= p else 0 (128x128)
    cmask_d = const.tile([128, 128], BF16)
    nc.gpsimd.memset(cmask_d[:], 1.0)
    nc.gpsimd.affine_select(
        out=cmask_d[:], in_=cmask_d[:], compare_op=ALU.is_ge, fill=0.0,
        base=0, channel_multiplier=-1, pattern=[[1, 128]],
    )
    # register const APs used as activation biases (Exp/Sigmoid need 0.0,
    # Sqrt uses EPS); bass converts float biases via nc.const_aps
    for val in (0.0, EPS / (AS * AS)):
        cz = const.tile([128, 1], F32, name=f"constap_{val}")
        nc.vector.memset(cz[:], val)
        nc.const_aps.aps[(F32, val)] = cz[:]
    wr_b = const.tile([128, C], F32)
    nc.scalar.dma_start(out=wr_b[0:1, :], in_=wr_d[:, :])
    nc.gpsimd.partition_broadcast(wr_b[:], wr_b[0:1, :])
    iota_f = const.tile([128, 1], F32)          # partition index 0..127
    iota_i = const.tile([128, 1], I32)
    nc.gpsimd.iota(iota_i[:], pattern=[[1, 1]], base=0, channel_multiplier=1)
    nc.vector.tensor_copy(iota_f[:], iota_i[:])
    iota16_i = const.tile([16, 256], I32)       # token-index iota (16-wrap)
    nc.gpsimd.iota(iota16_i[:], pattern=[[16, 256]], base=1,
                   channel_multiplier=1)
    iota16_f = const.tile([16, 256], F32)
    nc.vector.tensor_copy(iota16_f[:], iota16_i[:])
    logit_sb = const.tile([128, NT], F32)       # token t = col*128 + p

    # persistent activations
    py = es.enter_context(tc.tile_pool(name="py", bufs=1))
    y0 = py.tile([128, NI, C], F32)             # gathered rows; swr reuses it
    y1 = py.tile([128, NI, C], F32)             # after attention residual
    idx128 = py.tile([128, 32], I16)
    w128 = py.tile([128, NI], F32)

    # ---------------- stage 1: x load, logits, x copy-through --------------
    with tc.tile_pool(name="xio", bufs=8) as xio, \
         tc.tile_pool(name="junkp", bufs=2) as junkp:
        for t in range(NT):
            xt = xio.tile([128, C], F32, tag="xt")
            nc.sync.dma_start(out=xt[:], in_=x_d[t * 128:(t + 1) * 128, :])
            junk = junkp.tile([128, C], BF16, tag="junk")
            nc.vector.scalar_tensor_tensor(
                out=junk[:], in0=xt[:], scalar=1.0, in1=wr_b[:],
                op0=ALU.mult, op1=ALU.mult,
                accum_out=logit_sb[:, t:t + 1])

    if KSTOP == 1:
        nc.scalar.dma_start(out=out_d[0:128, 0:NT], in_=logit_sb[:])
        return

    # ---------------- stage 2: routing (exact 512th-largest) --------------
    with tc.tile_pool(name="route", bufs=1) as rt, \
         tc.tile_pool(name="rpsum", bufs=2, space="PSUM") as rpsum, \
         tc.tile_pool(name="hpsum", bufs=1, space="PSUM") as hpsum:
        # replicate all 4096 logits to every partition (f32-exact; the
        # counting search is order-agnostic so any permutation is fine)
        lrow = rt.tile([1, T], F32)
        nc.scalar.dma_start(out=lrow[:], in_=logit_sb[:])
        lrep = rt.tile([128, T], F32)
        nc.gpsimd.partition_broadcast(lrep[:], lrow[:])

        # l16 rearrangement for the final compaction (independent of thr;
        # on the gpsimd SWDGE queue so it overlaps the DVE count stages)
        l16 = rt.tile([16, 256], F32)
        for g in range(8):
            nc.gpsimd.dma_start(out=l16[:, g::8],
                                in_=logit_sb[g * 16:(g + 1) * 16, :])

        t_lo = rt.tile([128, 1], F32)
        delta = rt.tile([128, 1], F32)
        theta = rt.tile([128, 1], F32)
        cnt = rt.tile([128, 1], F32)
        flag = rt.tile([128, 1], BF16)
        s_sb = rt.tile([1, 1], BF16)
        step = rt.tile([128, 1], F32)
        cjunk = rt.tile([128, T], BF16)
        heat_src = rt.tile([128, 512], BF16)
        nc.vector.memset(heat_src[:], 1.0)
        flag_h = [rt.tile([128, 1], BF16, name=f"flag_h{s}")
                  for s in range(4)]
        nc.vector.memset(t_lo[:], -4.0)
        nc.vector.memset(delta[:], 8.0 / 128.0)
        # theta = iota*delta + t_lo
        nc.vector.scalar_tensor_tensor(
            out=theta[:], in0=iota_f[:], scalar=delta[:, 0:1], in1=t_lo[:],
            op0=ALU.mult, op1=ALU.add)
        for st in range(4):
            nc.vector.tensor_scalar(
                out=cjunk[:], in0=lrep[:], scalar1=theta[:, 0:1], scalar2=None,
                op0=ALU.is_ge, op1=ALU.add, accum_out=cnt[:])
            nc.vector.tensor_scalar(
                out=flag[:], in0=cnt[:], scalar1=float(K) - 0.5, scalar2=None,
                op0=ALU.is_ge)
            sp = rpsum.tile([1, 1], F32, tag="sp")
            nc.tensor.matmul(sp[:], flag[:], ones_col[:], start=True, stop=True)
            nc.vector.tensor_copy(s_sb[:], sp[:])
            srep = rpsum.tile([128, 1], F32, tag="srep")
            nc.tensor.matmul(srep[:], ones_row[:], s_sb[:], start=True,
                             stop=True)
            # t_lo += (s-1)*delta ; delta /= 128 ; theta = iota*delta + t_lo
            nc.vector.scalar_tensor_tensor(
                out=step[:], in0=srep[:], scalar=-1.0, in1=delta[:],
                op0=ALU.add, op1=ALU.mult)
            nc.vector.tensor_tensor(out=t_lo[:], in0=t_lo[:], in1=step[:],
                                    op=ALU.add)
            # PE heater: junk matmuls anchored on this stage's flag keep the
            # HAM clock-gate at 8/8 through the routing latency chain, so the
            # block starts warm (cold PE runs at half clock for ~3.4us)
            nc.vector.tensor_copy(flag_h[st][:], flag[:])
            nheat = 40 if st == 3 else 26
            for _k in range(nheat):
                hp_ = hpsum.tile([1, 512], F32, tag="hp")
                nc.tensor.matmul(hp_[:], flag_h[st][:], heat_src[:],
                                 start=True, stop=True)
            if st < 3:
                nc.vector.tensor_scalar_mul(delta[:], delta[:], 1.0 / 128.0)
                nc.vector.scalar_tensor_tensor(
                    out=theta[:], in0=iota_f[:], scalar=delta[:, 0:1],
                    in1=t_lo[:], op0=ALU.mult, op1=ALU.add)

        # mask + ascending-index compaction (as before, thr = t_lo)
        m01 = rt.tile([16, 256], F32)
        nc.vector.tensor_scalar(out=m01[:], in0=l16[:],
                                scalar1=t_lo[0:16, 0:1],
                                scalar2=None, op0=ALU.is_ge)
        selm = rt.tile([16, 256], F32)   # j+1 if selected else 0 ... then -1
        nc.vector.tensor_tensor(out=selm[:], in0=m01[:], in1=iota16_f[:],
                                op=ALU.mult)
        nc.vector.tensor_scalar_add(selm[:], selm[:], -1.0)
        wcand = rt.tile([16, 256], F32)  # logit+99 if selected else -1
        nc.vector.scalar_tensor_tensor(out=wcand[:], in0=l16[:], scalar=100.0,
                                       in1=m01[:], op0=ALU.add, op1=ALU.mult)
        nc.vector.tensor_scalar_add(wcand[:], wcand[:], -1.0)

        idxw = rt.tile([16, 32], F32)
        wsel = rt.tile([16, 32], F32)
        nfound = rt.tile([1, 1], U32)
        nfound2 = rt.tile([1, 1], U32)
        nc.gpsimd.sparse_gather(idxw[:], selm[:], num_found=nfound[:])
        nc.gpsimd.sparse_gather(wsel[:], wcand[:], num_found=nfound2[:])
        nc.vector.tensor_scalar_add(wsel[:], wsel[:], -99.0)
        idxw16 = rt.tile([16, 32], I16)
        nc.vector.tensor_copy(idxw16[:], idxw[:])

        # distribute into gather layout (SBUF->SBUF; idx128 group g is a
        # straight copy of idxw16, w128[16g+q, i] = wsel[q, 8i+g])
        for g in range(8):
            nc.scalar.dma_start(out=idx128[g * 16:(g + 1) * 16, :],
                                in_=idxw16[:])
        for g in range(8):
            nc.scalar.dma_start(out=w128[g * 16:(g + 1) * 16, :],
                                in_=wsel[:, g::8])
        # copy-through out=x, DRAM->DRAM on the scalar ring: drains through
        # the routing/attention window without touching SBUF or the weights
        for q in range(4):
            nc.scalar.dma_start(out=out_d[q * 1024:(q + 1) * 1024, :],
                                in_=x_d[q * 1024:(q + 1) * 1024, :])

    if KSTOP == 2:
        nc.scalar.dma_start(out=out_d[0:128, 0:1], in_=w128[:, 0:1])
        return

    # ---------------- stage 3: gather + LN1 + transpose ----------------
    nc.gpsimd.dma_gather(out_ap=y0[:], in_ap=x_d[:, :], idxs_ap=idx128[:],
                         num_idxs=K, num_idxs_reg=K, elem_size=C)

    if KSTOP == 3:
        for c in range(NI):
            nc.sync.dma_start(out=out_d[c * 128:(c + 1) * 128, :],
                              in_=y0[:, c, :])
        nc.sync.dma_start(out=out_d[512:640, 0:NI], in_=w128[:])
        return

    lnp = es.enter_context(tc.tile_pool(name="lnp", bufs=4))

    def ln_tokmajor(src, dst):
        # LayerNorm over free dim (C) of token-major [128, C] f32 -> bf16,
        # output scaled by AS (=16) for the fp8 pipeline
        st = lnp.tile([128, 2, 6], F32, tag="bnst")
        nc.vector.bn_stats(st[:, 0, :], src[:, 0:512])
        nc.vector.bn_stats(st[:, 1, :], src[:, 512:1024])
        ag = lnp.tile([128, 2], F32, tag="bnag")
        nc.vector.bn_aggr(ag[:], st[:])
        sd = lnp.tile([128, 1], F32, tag="sd")
        nc.scalar.activation(sd[:], ag[:, 1:2], AF.Sqrt, bias=EPS / (AS * AS),
                             scale=1.0 / (AS * AS))
        rs16 = lnp.tile([128, 1], F32, tag="rs16")
        nc.vector.reciprocal(rs16[:], sd[:])
        nb = lnp.tile([128, 1], F32, tag="nb")
        nc.vector.scalar_tensor_tensor(out=nb[:], in0=ag[:, 0:1], scalar=-1.0,
                                       in1=rs16[:], op0=ALU.mult, op1=ALU.mult)
        nc.scalar.activation(dst[:], src[:], AF.Identity, bias=nb[:],
                             scale=rs16[:])

    from contextlib import ExitStack as _ES
    pq_stack = _ES()
    pqkv = pq_stack.enter_context(tc.tile_pool(name="pqkv", bufs=1))
    hT8 = pqkv.tile([128, NCC, K], FP8, name="hT8")
    qkv_sb = [pqkv.tile([128, K], BF16, name=f"qkv{m}") for m in range(24)]
    o2 = [pqkv.tile([128, K], BF16, name=f"o2_{hp}") for hp in range(8)]
    # weight prefetch: the sync ring is idle during routing, so these drain
    # right after the x reads, well before their consuming stages
    wq = []
    for jp in range(NCP):
        wt = pqkv.tile([128, 2, 3 * C], FP8, name=f"wq{jp}")
        for i in range(2):
            nc.sync.dma_start(
                out=wt[:, i, :],
                in_=wqkv_d[(2 * jp + i) * 128:(2 * jp + i + 1) * 128, :])
        wq.append(wt)
    wo_sb = []
    for hp in range(8):
        wt = pqkv.tile([128, C], BF16, name=f"wo{hp}")
        nc.sync.dma_start(out=wt[:], in_=wo_d[hp * 128:(hp + 1) * 128, :])
        wo_sb.append(wt)

    with tc.tile_pool(name="hbuf", bufs=2) as hbuf, \
         tc.tile_pool(name="tpsum", bufs=4, space="PSUM") as tpsum:
        for i in range(NI):
            hti = hbuf.tile([128, C], BF16, tag="h")
            ln_tokmajor(y0[:, i, :], hti[:])
            for cc in range(NCC):
                pt = tpsum.tile([128, 128], BF16, tag="tp")
                nc.tensor.transpose(pt[:], hti[:, cc * 128:(cc + 1) * 128],
                                    ident[:])
                nc.vector.tensor_copy(
                    hT8[:, cc, i * 128:(i + 1) * 128], pt[:])

    # ---------------- stage 4: QKV projection (fp8 DoubleRow) -------------
    with tc.tile_pool(name="qpsum", bufs=3, space="PSUM") as qpsum:
        m_order = []
        for qc in range(8):
            m_order += [16 + qc, qc, 8 + qc]
        for m in m_order:
            pq = qpsum.tile([128, K], F32, tag="pq")
            for jp in range(NCP):
                nc.tensor.matmul(pq[:],
                                 wq[jp][:, :, m * 128:(m + 1) * 128],
                                 hT8[:, 2 * jp:2 * jp + 2, :],
                                 start=(jp == 0), stop=(jp == NCP - 1),
                                 perf_mode=DR)
            nc.vector.tensor_scalar_mul(qkv_sb[m][:], pq[:], INV)

    if KSTOP == 4:
        for m in range(24):
            nc.gpsimd.dma_start(out=out_d[m * 128:(m + 1) * 128, 0:K],
                                in_=qkv_sb[m][:])
        return

    # ---------------- stage 5: attention ----------------
    # layouts: q = qkv chunks 0-7, k = 8-15, v = 16-23; head h lives in chunk
    # h//2 at partition offset 64*(h%2). o2 holds RAW (unnormalized) head
    # outputs; softmax denominators are batch-reciprocaled and divided in at
    # the end (single-lane reciprocals are ~3.4us each on DVE, batching puts
    # all 16 heads in one per-lane pass).
    zall = pq_stack.enter_context(tc.tile_pool(name="zp", bufs=2))
    with tc.tile_pool(name="apool", bufs=8) as ap_, \
         tc.tile_pool(name="vtep", bufs=10) as vtep, \
         tc.tile_pool(name="praw", bufs=6) as praw, \
         tc.tile_pool(name="spsum", bufs=2, space="PSUM") as spsum, \
         tc.tile_pool(name="vpsum", bufs=2, space="PSUM") as vpsum, \
         tc.tile_pool(name="zpsum", bufs=1, space="PSUM") as zpsum, \
         tc.tile_pool(name="opsum", bufs=3, space="PSUM") as opsum:
        zs64 = None
        raw_odd = {}
        for qch in range(8):
            if qch % 4 == 0:
                # fresh Z-strip container per batch of 8 heads
                zs64 = zall.tile([65, 8, K], BF16, tag="zs64")
            v_pair = qkv_sb[16 + qch]
            vte_jc = []
            for jc in range(NI):
                pv = vpsum.tile([128, 128], BF16, tag="pv")
                nc.tensor.transpose(pv[:], v_pair[:, jc * 128:(jc + 1) * 128],
                                    ident[:])
                vte = vtep.tile([128, 2, 65], BF16, tag="vte")
                nc.vector.tensor_copy(
                    vte[:, :, 0:64],
                    pv[:].rearrange("p (two f) -> p two f", two=2))
                nc.vector.memset(vte[:, :, 64:65], 1.0)
                vte_jc.append(vte)
            for h2 in range(2):
                h = 2 * qch + h2
                po = 64 * h2
                q_sl = qkv_sb[qch][po:po + 64, :]
                k_sl = qkv_sb[8 + qch][po:po + 64, :]
                att = []
                for jc in range(NI):
                    # causal skip: queries i < jc*128 are fully masked for
                    # this j-chunk; only the diagonal block needs the mask
                    lo = jc * 128
                    ps = spsum.tile([128, K], F32, tag="ps")
                    nc.tensor.matmul(ps[:, lo:],
                                     k_sl[:, jc * 128:(jc + 1) * 128],
                                     q_sl[:, lo:], start=True, stop=True)
                    am = ap_.tile([128, K], BF16, tag="am")
                    nc.scalar.activation(am[:, lo:], ps[:, lo:], AF.Exp,
                                         scale=0.125)
                    nc.vector.tensor_tensor(out=am[:, lo:lo + 128],
                                            in0=am[:, lo:lo + 128],
                                            in1=cmask_d[:], op=ALU.mult)
                    att.append(am)
                po_t = opsum.tile([65, K], F32, tag="po")
                for jc in range(NI):
                    lo = jc * 128
                    nc.tensor.matmul(po_t[:, lo:], vte_jc[jc][:, h2, 0:65],
                                     att[jc][:, lo:],
                                     start=(jc == 0), stop=(jc == 3))
                # evict raw o + Z strip (lane-aligned); PSUM freed here
                if h % 2 == 0:
                    nc.vector.tensor_copy(o2[qch][0:64, :], po_t[0:64, :])
                else:
                    orw = praw.tile([64, K], BF16, tag="orw")
                    raw_odd[h] = orw
                    nc.vector.tensor_copy(orw[:], po_t[0:64, :])
                nc.scalar.activation(zs64[64:65, h % 8, :], po_t[64:65, :],
                                     AF.Copy)
                if h % 8 == 7:
                    # shuffle 8 Z strips onto partitions 0..7, batch-
                    # reciprocal, bounce back to partition 0; per head,
                    # broadcast 1/Z across 64 partitions with a tiny PE
                    # matmul (lane-aligned) and divide into the raw output
                    hs = h - 7
                    ztb = zall.tile([8, K], BF16, tag="ztb")
                    nc.sync.dma_start(out=ztb[:], in_=zs64[64:65, :, :])
                    zrf = zall.tile([8, K], F32, tag="zrf")
                    nc.vector.reciprocal(zrf[:], ztb[:])
                    zrb = zall.tile([8, K], BF16, tag="zrb")
                    nc.vector.tensor_copy(zrb[:], zrf[:])
                    zrbT = zall.tile([1, 8, K], BF16, tag="zrbT")
                    nc.sync.dma_start(out=zrbT[:], in_=zrb[:])
                    for hh in range(hs, hs + 8):
                        pz = zpsum.tile([64, K], F32, tag="pz")
                        nc.tensor.matmul(pz[:], ones_row[0:1, 0:64],
                                         zrbT[0:1, hh - hs, :],
                                         start=True, stop=True)
                        if hh % 2 == 0:
                            nc.vector.tensor_tensor(
                                out=o2[hh // 2][0:64, :],
                                in0=o2[hh // 2][0:64, :], in1=pz[:],
                                op=ALU.mult)
                        else:
                            orw = raw_odd.pop(hh)
                            nc.vector.tensor_tensor(
                                out=orw[:], in0=orw[:], in1=pz[:],
                                op=ALU.mult)
                            nc.sync.dma_start(
                                out=o2[hh // 2][64:128, :], in_=orw[:])

    if KSTOP == 5:
        for hp in range(8):
            nc.gpsimd.dma_start(out=out_d[hp * 128:(hp + 1) * 128, 0:K],
                                in_=o2[hp][:])
        return

    # ---------------- stage 6: Wo (paired heads) + residual ----------------
    with tc.tile_pool(name="aopsum", bufs=2, space="PSUM") as aopsum:
        for i in range(NI):
            pao = aopsum.tile([128, C], F32, tag="pao")
            for hp in range(8):
                for nh in range(2):
                    nc.tensor.matmul(
                        pao[:, nh * 512:(nh + 1) * 512],
                        o2[hp][:, i * 128:(i + 1) * 128],
                        wo_sb[hp][:, nh * 512:(nh + 1) * 512],
                        start=(hp == 0), stop=(hp == 7))
            nc.vector.tensor_tensor(out=y1[:, i, :], in0=pao[:],
                                    in1=y0[:, i, :], op=ALU.add)

    if KSTOP == 6:
        for c in range(NI):
            nc.sync.dma_start(out=out_d[c * 128:(c + 1) * 128, :],
                              in_=y1[:, c, :])
        return
    pq_stack.close()

    # ---------------- stage 7: LN2 + transpose ----------------
    pmlp = es.enter_context(tc.tile_pool(name="pmlp", bufs=1))
    mT8 = pmlp.tile([128, NCC, K], FP8, name="mT8")
    h2big = pmlp.tile([128, NFC, K], FP8, name="h2big")
    w2_sb = []
    for fp in range(16):
        w2t = pmlp.tile([128, 2, C], FP8, name=f"w2_{fp}")
        for i in range(2):
            r = (2 * fp + i) * 128
            nc.sync.dma_start(out=w2t[:, i, :], in_=w2_d[r:r + 128, :])
        w2_sb.append(w2t)
    with tc.tile_pool(name="mbuf", bufs=2) as mbuf, \
         tc.tile_pool(name="tpsum2", bufs=4, space="PSUM") as tpsum2:
        for i in range(NI):
            mti = mbuf.tile([128, C], BF16, tag="m")
            ln_tokmajor(y1[:, i, :], mti[:])
            for cc in range(NCC):
                pt = tpsum2.tile([128, 128], BF16, tag="tp2")
                nc.tensor.transpose(pt[:], mti[:, cc * 128:(cc + 1) * 128],
                                    ident[:])
                nc.vector.tensor_copy(
                    mT8[:, cc, i * 128:(i + 1) * 128], pt[:])

    # ---------------- stage 8: W1/W3 (fp8 DoubleRow) + SwiGLU -------------
    with tc.tile_pool(name="w13p", bufs=2) as w13p, \
         tc.tile_pool(name="upsum", bufs=2, space="PSUM") as upsum, \
         tc.tile_pool(name="gpsum", bufs=2, space="PSUM") as gpsum, \
         tc.tile_pool(name="sbuf8", bufs=3) as sbuf8:
        for fg in range(4):               # groups of 8 ffn chunks
            w1g, w3g = [], []
            for jp in range(NCP):
                t1 = w13p.tile([128, 2, 1024], FP8, tag=f"w1g{jp}")
                t3 = w13p.tile([128, 2, 1024], FP8, tag=f"w3g{jp}")
                for i in range(2):
                    r = (2 * jp + i) * 128
                    nc.sync.dma_start(
                        out=t1[:, i, :],
                        in_=w1_d[r:r + 128, fg * 1024:(fg + 1) * 1024])
                    nc.sync.dma_start(
                        out=t3[:, i, :],
                        in_=w3_d[r:r + 128, fg * 1024:(fg + 1) * 1024])
                w1g.append(t1)
                w3g.append(t3)
            for fi in range(8):
                f = fg * 8 + fi
                pu = upsum.tile([128, K], F32, tag="pu")
                pg = gpsum.tile([128, K], F32, tag="pg")
                for jp in range(NCP):
                    nc.tensor.matmul(pu[:],
                                     w1g[jp][:, :, fi * 128:(fi + 1) * 128],
                                     mT8[:, 2 * jp:2 * jp + 2, :],
                                     start=(jp == 0), stop=(jp == NCP - 1),
                                     perf_mode=DR)
                for jp in range(NCP):
                    nc.tensor.matmul(pg[:],
                                     w3g[jp][:, :, fi * 128:(fi + 1) * 128],
                                     mT8[:, 2 * jp:2 * jp + 2, :],
                                     start=(jp == 0), stop=(jp == NCP - 1),
                                     perf_mode=DR)
                # u = pu * INV;  h2 (scaled x16 for fp8) = us * pg * INV*16
                sg = sbuf8.tile([128, K], BF16, tag="sg")
                nc.scalar.activation(sg[:], pu[:], AF.Sigmoid, scale=INV)
                us = sbuf8.tile([128, K], BF16, tag="us")
                nc.vector.scalar_tensor_tensor(out=us[:], in0=pu[:],
                                               scalar=INV, in1=sg[:],
                                               op0=ALU.mult, op1=ALU.mult)
                nc.vector.scalar_tensor_tensor(out=h2big[:, f, :], in0=pg[:],
                                               scalar=INV * AS, in1=us[:],
                                               op0=ALU.mult, op1=ALU.mult)

    if KSTOP == 8:
        with tc.tile_pool(name="dbg8", bufs=2) as dbg8:
            for f in range(8):
                db = dbg8.tile([128, K], F32, tag="db")
                nc.vector.tensor_copy(db[:], h2big[:, f, :])
                nc.sync.dma_start(out=out_d[f * 128:(f + 1) * 128, 0:K],
                                  in_=db[:])
        return

    # ---------------- stage 9: W2 (fp8 DoubleRow) + residual + w-scale ----
    # W2 is fully resident (prefetched at stage 7); per-chunk scatter-add
    # overlaps the remaining chunks' matmuls
    with tc.tile_pool(name="mpsum", bufs=2, space="PSUM") as mpsum, \
         tc.tile_pool(name="y2buf", bufs=2) as y2buf:
        for i in range(NI):
            pm = mpsum.tile([128, C], F32, tag="pm")
            for fp in range(16):
                for nh in range(2):
                    nc.tensor.matmul(
                        pm[:, nh * 512:(nh + 1) * 512],
                        h2big[:, 2 * fp:2 * fp + 2, i * 128:(i + 1) * 128],
                        w2_sb[fp][:, :, nh * 512:(nh + 1) * 512],
                        start=(fp == 0), stop=(fp == 15),
                        perf_mode=DR)
            y2t = y2buf.tile([128, C], F32, tag="y2")
            nc.vector.scalar_tensor_tensor(out=y2t[:], in0=pm[:],
                                           scalar=INV, in1=y1[:, i, :],
                                           op0=ALU.mult, op1=ALU.add)
            # swr reuses y0's storage (y0 is dead after stage 6)
            nc.scalar.activation(y0[:, i, :], y2t[:], AF.Copy,
                                 scale=w128[:, i:i + 1])
            nc.gpsimd.dma_scatter_add(out_ap=out_d[:, :],
                                      in_ap=y0[:, i:i + 1, :],
                                      idxs_ap=idx128[:, 8 * i:8 * i + 8],
                                      num_idxs=128, num_idxs_reg=128,
                                      elem_size=C)


_CACHE = {}


def _get_compiled():
    if "nc" in _CACHE:
        return _CACHE["nc"]
    from contextlib import ExitStack
    nc = bacc.Bacc("TRN2", target_bir_lowering=False, debug=False)
    with tile.TileContext(nc) as tc:
        with ExitStack() as es:
            build(nc, tc, es)
    nc.compile()
    _CACHE["nc"] = nc
    return nc


def _prep_host_inputs(inputs):
    x = np.asarray(inputs["x"], dtype=np.float32)          # (8, 4096, 1024)
    Wr = np.asarray(inputs["Wr"], dtype=np.float32)
    ln1_g = np.asarray(inputs["ln1_g"], dtype=np.float32)
    ln2_g = np.asarray(inputs["ln2_g"], dtype=np.float32)
    f8 = ml_dtypes.float8_e4m3fn

    def q8(a):
        return np.clip(a * WS, -240.0, 240.0).astype(f8)

    wqkv = q8(np.asarray(inputs["Wqkv"], np.float32) * ln1_g[:, None])
    wo = np.asarray(inputs["Wo"], np.float32).astype(ml_dtypes.bfloat16)
    w1 = q8(np.asarray(inputs["W1"], np.float32) * ln2_g[:, None])
    w3 = q8(np.asarray(inputs["W3"], np.float32) * ln2_g[:, None])
    w2 = q8(np.asarray(inputs["W2"], np.float32))
    shared = {
        "wr": np.ascontiguousarray(Wr[None, :]),
        "wqkv": np.ascontiguousarray(wqkv),
        "wo": np.ascontiguousarray(wo),
        "w1": np.ascontiguousarray(w1),
        "w3": np.ascontiguousarray(w3),
        "w2": np.ascontiguousarray(w2),
    }
    return [{"x": np.ascontiguousarray(x[b]), **shared} for b in range(B)]


def kernel(**inputs):
    nc = _get_compiled()
    in_maps = _prep_host_inputs(inputs)
    res = run_bass_kernel_spmd(nc, in_maps, core_ids=list(range(N_CORES)))
    _CACHE["last_results"] = res
    out = np.stack([res.results[b]["out"] for b in range(B)], axis=0)
    return out.astype(np.float32)


# revision 33
# speedup vs baseline: 1.0388x; 1.0388x over previous
"""Trainium2 Bass kernel for nn_MoDBlock (mixture-of-depths transformer block).

Sharding: data-parallel over batch B=8 across the 8 NeuronCores (one batch row
per core; routing/gather/scatter are per-row independent). Everything runs
on-device per core:

  logits  = x @ Wr                      (DVE fused mul+reduce, 32 tiles)
  thr     = exact 512th-largest logit via 5-stage counting search:
            128 candidate thresholds/stage on DVE (is_ge + accum count),
            flag-sum via PE matmul, interval refined x128 per stage down to
            < 1 ULP -> threshold t with #(logits >= t) == 512 exactly.
  sel, w  = ascending-index compaction  (gpsimd sparse_gather on masked iota /
                                         shifted logits)
  tok     = dma_gather(x, sel)          (512 rows of 4KB)
  block   = pre-LN attention + SwiGLU MLP; fp8e4 DoubleRow matmuls (2x PE
            throughput) for QKV/W1/W3/W2 with f32 accumulation; weights
            pre-scaled x64 and LN outputs x16 on-chip (fp8 denormal
            avoidance), rescaled 2^-10 at PSUM eviction; bf16 attention;
            f32 LN/softmax statistics; softmax without max-subtraction
            (|scores/8| < 3 at this operator's scale), causal mask applied
            multiplicatively only on the diagonal 128x128 block.
  out     = copy of x (written from the stage-1 tiles; x is read once),
            then dma_scatter_add(out, proc * w, sel)

Host-side preprocessing: weights cast to fp8e4 (x64, clipped +-240) / Wo to
bf16, LN gains folded into Wqkv/W1/W3 rows.
"""

import os
import numpy as np
import ml_dtypes

import concourse.bass as bass
import concourse.mybir as mybir
import concourse.tile as tile
from concourse import bacc, masks
from concourse.bass_utils import run_bass_kernel_spmd

F32 = mybir.dt.float32
BF16 = mybir.dt.bfloat16
FP8 = mybir.dt.float8e4
I16 = mybir.dt.int16
I32 = mybir.dt.int32
U32 = mybir.dt.uint32
AF = mybir.ActivationFunctionType
ALU = mybir.AluOpType
DR = mybir.MatmulPerfMode.DoubleRow

B, T, C = 8, 4096, 1024
H, DH, FF = 16, 64, 4096
K = 512                      # routed tokens per batch row
EPS = 1e-5
NT = T // 128                # 32 x-tiles
NI = K // 128                # 4 reduced-seq token chunks
NCC = C // 128               # 8 feature chunks
NCP = NCC // 2               # 4 feature pair-chunks (DoubleRow)
NFC = FF // 128              # 32 ffn chunks
N_CORES = 8

WS = 64.0                    # host-side weight scale (fp8)
AS = 16.0                    # on-chip LN-output scale (fp8)
INV = 1.0 / (WS * AS)        # PSUM rescale 2^-10

KSTOP = int(os.environ.get("KSTOP", "99"))


def build(nc, tc, es):
    x_d = nc.dram_tensor("x", (T, C), F32, kind="ExternalInput").ap()
    wr_d = nc.dram_tensor("wr", (1, C), F32, kind="ExternalInput").ap()
    wqkv_d = nc.dram_tensor("wqkv", (C, 3 * C), FP8, kind="ExternalInput").ap()
    wo_d = nc.dram_tensor("wo", (C, C), BF16, kind="ExternalInput").ap()
    w1_d = nc.dram_tensor("w1", (C, FF), FP8, kind="ExternalInput").ap()
    w3_d = nc.dram_tensor("w3", (C, FF), FP8, kind="ExternalInput").ap()
    w2_d = nc.dram_tensor("w2", (FF, C), FP8, kind="ExternalInput").ap()
    out_d = nc.dram_tensor("out", (T, C), F32, kind="ExternalOutput").ap()

    const = es.enter_context(tc.tile_pool(name="const", bufs=1))
    ident = const.tile([128, 128], BF16)
    masks.make_identity(nc, ident[:])
    ones_col = const.tile([128, 1], BF16)       # all-ones column
    nc.vector.memset(ones_col[:], 1.0)
    ones_row = const.tile([1, 128], BF16)       # all-ones row on partition 0
    nc.vector.memset(ones_row[:], 1.0)
    # diagonal causal mask: cmask_d[p, q] = 1.0 if q >= p else 0 (128x128)
    cmask_d = const.tile([128, 128], BF16)
    nc.gpsimd.memset(cmask_d[:], 1.0)
    nc.gpsimd.affine_select(
        out=cmask_d[:], in_=cmask_d[:], compare_op=ALU.is_ge, fill=0.0,
        base=0, channel_multiplier=-1, pattern=[[1, 128]],
    )
    # register const APs used as activation biases (Exp/Sigmoid need 0.0,
    # Sqrt uses EPS); bass converts float biases via nc.const_aps
    for val in (0.0, EPS / (AS * AS)):
        cz = const.tile([128, 1], F32, name=f"constap_{val}")
        nc.vector.memset(cz[:], val)
        nc.const_aps.aps[(F32, val)] = cz[:]
    wr_b = const.tile([128, C], F32)
    nc.scalar.dma_start(out=wr_b[0:1, :], in_=wr_d[:, :])
    nc.gpsimd.partition_broadcast(wr_b[:], wr_b[0:1, :])
    iota_f = const.tile([128, 1], F32)          # partition index 0..127
    iota_i = const.tile([128, 1], I32)
    nc.gpsimd.iota(iota_i[:], pattern=[[1, 1]], base=0, channel_multiplier=1)
    nc.vector.tensor_copy(iota_f[:], iota_i[:])
    iota16_i = const.tile([16, 256], I32)       # token-index iota (16-wrap)
    nc.gpsimd.iota(iota16_i[:], pattern=[[16, 256]], base=1,
                   channel_multiplier=1)
    iota16_f = const.tile([16, 256], F32)
    nc.vector.tensor_copy(iota16_f[:], iota16_i[:])
    logit_sb = const.tile([128, NT], F32)       # token t = col*128 + p

    # persistent activations
    py = es.enter_context(tc.tile_pool(name="py", bufs=1))
    y0 = py.tile([128, NI, C], F32)             # gathered rows; swr reuses it
    y1 = py.tile([128, NI, C], F32)             # after attention residual
    idx128 = py.tile([128, 32], I16)
    w128 = py.tile([128, NI], F32)

    # ---------------- stage 1: x load, logits, x copy-through --------------
    with tc.tile_pool(name="xio", bufs=16) as xio, \
         tc.tile_pool(name="junkp", bufs=4) as junkp:
        for t in range(NT):
            xt = xio.tile([128, C], F32, tag="xt")
            nc.sync.dma_start(out=xt[:], in_=x_d[t * 128:(t + 1) * 128, :])
            junk = junkp.tile([128, C], BF16, tag="junk")
            nc.vector.scalar_tensor_tensor(
                out=junk[:], in0=xt[:], scalar=1.0, in1=wr_b[:],
                op0=ALU.mult, op1=ALU.mult,
                accum_out=logit_sb[:, t:t + 1])

    if KSTOP == 1:
        nc.scalar.dma_start(out=out_d[0:128, 0:NT], in_=logit_sb[:])
        return

    # ---------------- stage 2: routing (exact 512th-largest) --------------
    with tc.tile_pool(name="route", bufs=1) as rt, \
         tc.tile_pool(name="rpsum", bufs=2, space="PSUM") as rpsum, \
         tc.tile_pool(name="hpsum", bufs=1, space="PSUM") as hpsum:
        # replicate all 4096 logits to every partition (f32-exact; the
        # counting search is order-agnostic so any permutation is fine).
        # Two halves so replication of tiles 0-15 overlaps the tail of the
        # x load.
        lrow = rt.tile([1, T], F32)
        lrep = rt.tile([128, T], F32)
        nc.scalar.dma_start(out=lrow[0:1, 0:2048], in_=logit_sb[:, 0:16])
        nc.gpsimd.partition_broadcast(lrep[:, 0:2048], lrow[0:1, 0:2048])
        nc.scalar.dma_start(out=lrow[0:1, 2048:T], in_=logit_sb[:, 16:32])
        nc.gpsimd.partition_broadcast(lrep[:, 2048:T], lrow[0:1, 2048:T])

        t_lo = rt.tile([128, 1], F32)
        delta = rt.tile([128, 1], F32)
        theta = rt.tile([128, 1], F32)
        cnt = rt.tile([128, 1], F32)
        flag = rt.tile([128, 1], BF16)
        s_sb = rt.tile([1, 1], BF16)
        step = rt.tile([128, 1], F32)
        cjunk = rt.tile([128, T], BF16)
        heat_src = rt.tile([128, 512], BF16)
        nc.vector.memset(heat_src[:], 1.0)
        flag_h = [rt.tile([128, 1], BF16, name=f"flag_h{s}")
                  for s in range(4)]
        nc.vector.memset(t_lo[:], -4.0)
        nc.vector.memset(delta[:], 8.0 / 128.0)
        # theta = iota*delta + t_lo
        nc.vector.scalar_tensor_tensor(
            out=theta[:], in0=iota_f[:], scalar=delta[:, 0:1], in1=t_lo[:],
            op0=ALU.mult, op1=ALU.add)
        for st in range(4):
            nc.vector.tensor_scalar(
                out=cjunk[:], in0=lrep[:], scalar1=theta[:, 0:1], scalar2=None,
                op0=ALU.is_ge, op1=ALU.add, accum_out=cnt[:])
            nc.vector.tensor_scalar(
                out=flag[:], in0=cnt[:], scalar1=float(K) - 0.5, scalar2=None,
                op0=ALU.is_ge)
            sp = rpsum.tile([1, 1], F32, tag="sp")
            nc.tensor.matmul(sp[:], flag[:], ones_col[:], start=True, stop=True)
            nc.vector.tensor_copy(s_sb[:], sp[:])
            srep = rpsum.tile([128, 1], F32, tag="srep")
            nc.tensor.matmul(srep[:], ones_row[:], s_sb[:], start=True,
                             stop=True)
            # t_lo += (s-1)*delta ; delta /= 128 ; theta = iota*delta + t_lo
            nc.vector.scalar_tensor_tensor(
                out=step[:], in0=srep[:], scalar=-1.0, in1=delta[:],
                op0=ALU.add, op1=ALU.mult)
            nc.vector.tensor_tensor(out=t_lo[:], in0=t_lo[:], in1=step[:],
                                    op=ALU.add)
            # PE heater: junk matmuls anchored on this stage's flag keep the
            # HAM clock-gate at 8/8 through the routing latency chain, so the
            # block starts warm (cold PE runs at half clock for ~3.4us)
            nc.vector.tensor_copy(flag_h[st][:], flag[:])
            nheat = 40 if st == 3 else 26
            for _k in range(nheat):
                hp_ = hpsum.tile([1, 512], F32, tag="hp")
                nc.tensor.matmul(hp_[:], flag_h[st][:], heat_src[:],
                                 start=True, stop=True)
            if st < 3:
                nc.vector.tensor_scalar_mul(delta[:], delta[:], 1.0 / 128.0)
                nc.vector.scalar_tensor_tensor(
                    out=theta[:], in0=iota_f[:], scalar=delta[:, 0:1],
                    in1=t_lo[:], op0=ALU.mult, op1=ALU.add)

        # l16 rearrangement for the final compaction (emitted after the
        # counts so the gpsimd queue prioritizes the lrep broadcast)
        l16 = rt.tile([16, 256], F32)
        for g in range(8):
            nc.gpsimd.dma_start(out=l16[:, g::8],
                                in_=logit_sb[g * 16:(g + 1) * 16, :])
        # mask + ascending-index compaction (as before, thr = t_lo)
        m01 = rt.tile([16, 256], F32)
        nc.vector.tensor_scalar(out=m01[:], in0=l16[:],
                                scalar1=t_lo[0:16, 0:1],
                                scalar2=None, op0=ALU.is_ge)
        selm = rt.tile([16, 256], F32)   # j+1 if selected else 0 ... then -1
        nc.vector.tensor_tensor(out=selm[:], in0=m01[:], in1=iota16_f[:],
                                op=ALU.mult)
        nc.vector.tensor_scalar_add(selm[:], selm[:], -1.0)
        wcand = rt.tile([16, 256], F32)  # logit+99 if selected else -1
        nc.vector.scalar_tensor_tensor(out=wcand[:], in0=l16[:], scalar=100.0,
                                       in1=m01[:], op0=ALU.add, op1=ALU.mult)
        nc.vector.tensor_scalar_add(wcand[:], wcand[:], -1.0)

        idxw = rt.tile([16, 32], F32)
        wsel = rt.tile([16, 32], F32)
        nfound = rt.tile([1, 1], U32)
        nfound2 = rt.tile([1, 1], U32)
        nc.gpsimd.sparse_gather(idxw[:], selm[:], num_found=nfound[:])
        nc.gpsimd.sparse_gather(wsel[:], wcand[:], num_found=nfound2[:])
        nc.vector.tensor_scalar_add(wsel[:], wsel[:], -99.0)
        idxw16 = rt.tile([16, 32], I16)
        nc.vector.tensor_copy(idxw16[:], idxw[:])

        # distribute into gather layout (SBUF->SBUF; idx128 group g is a
        # straight copy of idxw16, w128[16g+q, i] = wsel[q, 8i+g])
        for g in range(8):
            nc.scalar.dma_start(out=idx128[g * 16:(g + 1) * 16, :],
                                in_=idxw16[:])
        for g in range(8):
            nc.scalar.dma_start(out=w128[g * 16:(g + 1) * 16, :],
                                in_=wsel[:, g::8])
        # copy-through out=x, DRAM->DRAM on the scalar ring: drains through
        # the routing/attention window without touching SBUF or the weights
        for q in range(4):
            nc.scalar.dma_start(out=out_d[q * 1024:(q + 1) * 1024, :],
                                in_=x_d[q * 1024:(q + 1) * 1024, :])

    if KSTOP == 2:
        nc.scalar.dma_start(out=out_d[0:128, 0:1], in_=w128[:, 0:1])
        return

    # ---------------- stage 3: gather + LN1 + transpose ----------------
    nc.gpsimd.dma_gather(out_ap=y0[:], in_ap=x_d[:, :], idxs_ap=idx128[:],
                         num_idxs=K, num_idxs_reg=K, elem_size=C)

    if KSTOP == 3:
        for c in range(NI):
            nc.sync.dma_start(out=out_d[c * 128:(c + 1) * 128, :],
                              in_=y0[:, c, :])
        nc.sync.dma_start(out=out_d[512:640, 0:NI], in_=w128[:])
        return

    lnp = es.enter_context(tc.tile_pool(name="lnp", bufs=4))

    def ln_tokmajor(src, dst):
        # LayerNorm over free dim (C) of token-major [128, C] f32 -> bf16,
        # output scaled by AS (=16) for the fp8 pipeline
        st = lnp.tile([128, 2, 6], F32, tag="bnst")
        nc.vector.bn_stats(st[:, 0, :], src[:, 0:512])
        nc.vector.bn_stats(st[:, 1, :], src[:, 512:1024])
        ag = lnp.tile([128, 2], F32, tag="bnag")
        nc.vector.bn_aggr(ag[:], st[:])
        sd = lnp.tile([128, 1], F32, tag="sd")
        nc.scalar.activation(sd[:], ag[:, 1:2], AF.Sqrt, bias=EPS / (AS * AS),
                             scale=1.0 / (AS * AS))
        rs16 = lnp.tile([128, 1], F32, tag="rs16")
        nc.vector.reciprocal(rs16[:], sd[:])
        nb = lnp.tile([128, 1], F32, tag="nb")
        nc.vector.scalar_tensor_tensor(out=nb[:], in0=ag[:, 0:1], scalar=-1.0,
                                       in1=rs16[:], op0=ALU.mult, op1=ALU.mult)
        nc.scalar.activation(dst[:], src[:], AF.Identity, bias=nb[:],
                             scale=rs16[:])

    from contextlib import ExitStack as _ES
    pq_stack = _ES()
    pqkv = pq_stack.enter_context(tc.tile_pool(name="pqkv", bufs=1))
    hT8 = pqkv.tile([128, NCC, K], FP8, name="hT8")
    qkv_sb = [pqkv.tile([128, K], BF16, name=f"qkv{m}") for m in range(24)]
    o2 = [pqkv.tile([128, K], BF16, name=f"o2_{hp}") for hp in range(8)]
    # weight prefetch: the sync ring is idle during routing, so these drain
    # right after the x reads, well before their consuming stages
    wq = []
    for jp in range(NCP):
        wt = pqkv.tile([128, 2, 3 * C], FP8, name=f"wq{jp}")
        for i in range(2):
            nc.sync.dma_start(
                out=wt[:, i, :],
                in_=wqkv_d[(2 * jp + i) * 128:(2 * jp + i + 1) * 128, :])
        wq.append(wt)
    wo_sb = []
    for hp in range(8):
        wt = pqkv.tile([128, C], BF16, name=f"wo{hp}")
        nc.sync.dma_start(out=wt[:], in_=wo_d[hp * 128:(hp + 1) * 128, :])
        wo_sb.append(wt)

    with tc.tile_pool(name="hbuf", bufs=2) as hbuf, \
         tc.tile_pool(name="tpsum", bufs=4, space="PSUM") as tpsum:
        for i in range(NI):
            hti = hbuf.tile([128, C], BF16, tag="h")
            ln_tokmajor(y0[:, i, :], hti[:])
            for cc in range(NCC):
                pt = tpsum.tile([128, 128], BF16, tag="tp")
                nc.tensor.transpose(pt[:], hti[:, cc * 128:(cc + 1) * 128],
                                    ident[:])
                nc.vector.tensor_copy(
                    hT8[:, cc, i * 128:(i + 1) * 128], pt[:])

    # ---------------- stage 4: QKV projection (fp8 DoubleRow) -------------
    with tc.tile_pool(name="qpsum", bufs=3, space="PSUM") as qpsum:
        m_order = []
        for qc in range(8):
            m_order += [16 + qc, qc, 8 + qc]
        for m in m_order:
            pq = qpsum.tile([128, K], F32, tag="pq")
            for jp in range(NCP):
                nc.tensor.matmul(pq[:],
                                 wq[jp][:, :, m * 128:(m + 1) * 128],
                                 hT8[:, 2 * jp:2 * jp + 2, :],
                                 start=(jp == 0), stop=(jp == NCP - 1),
                                 perf_mode=DR)
            nc.vector.tensor_scalar_mul(qkv_sb[m][:], pq[:], INV)

    if KSTOP == 4:
        for m in range(24):
            nc.gpsimd.dma_start(out=out_d[m * 128:(m + 1) * 128, 0:K],
                                in_=qkv_sb[m][:])
        return

    # ---------------- stage 5: attention ----------------
    # layouts: q = qkv chunks 0-7, k = 8-15, v = 16-23; head h lives in chunk
    # h//2 at partition offset 64*(h%2). o2 holds RAW (unnormalized) head
    # outputs; softmax denominators are batch-reciprocaled and divided in at
    # the end (single-lane reciprocals are ~3.4us each on DVE, batching puts
    # all 16 heads in one per-lane pass).
    zall = pq_stack.enter_context(tc.tile_pool(name="zp", bufs=2))
    with tc.tile_pool(name="apool", bufs=8) as ap_, \
         tc.tile_pool(name="vtep", bufs=10) as vtep, \
         tc.tile_pool(name="praw", bufs=6) as praw, \
         tc.tile_pool(name="spsum", bufs=2, space="PSUM") as spsum, \
         tc.tile_pool(name="vpsum", bufs=2, space="PSUM") as vpsum, \
         tc.tile_pool(name="zpsum", bufs=1, space="PSUM") as zpsum, \
         tc.tile_pool(name="opsum", bufs=3, space="PSUM") as opsum:
        zs64 = None
        raw_odd = {}
        for qch in range(8):
            if qch % 4 == 0:
                # fresh Z-strip container per batch of 8 heads
                zs64 = zall.tile([65, 8, K], BF16, tag="zs64")
            v_pair = qkv_sb[16 + qch]
            vte_jc = []
            for jc in range(NI):
                pv = vpsum.tile([128, 128], BF16, tag="pv")
                nc.tensor.transpose(pv[:], v_pair[:, jc * 128:(jc + 1) * 128],
                                    ident[:])
                vte = vtep.tile([128, 2, 65], BF16, tag="vte")
                nc.vector.tensor_copy(
                    vte[:, :, 0:64],
                    pv[:].rearrange("p (two f) -> p two f", two=2))
                nc.vector.memset(vte[:, :, 64:65], 1.0)
                vte_jc.append(vte)
            for h2 in range(2):
                h = 2 * qch + h2
                po = 64 * h2
                q_sl = qkv_sb[qch][po:po + 64, :]
                k_sl = qkv_sb[8 + qch][po:po + 64, :]
                att = []
                for jc in range(NI):
                    # causal skip: queries i < jc*128 are fully masked for
                    # this j-chunk; only the diagonal block needs the mask
                    lo = jc * 128
                    ps = spsum.tile([128, K], F32, tag="ps")
                    nc.tensor.matmul(ps[:, lo:],
                                     k_sl[:, jc * 128:(jc + 1) * 128],
                                     q_sl[:, lo:], start=True, stop=True)
                    am = ap_.tile([128, K], BF16, tag="am")
                    nc.scalar.activation(am[:, lo:], ps[:, lo:], AF.Exp,
                                         scale=0.125)
                    nc.vector.tensor_tensor(out=am[:, lo:lo + 128],
                                            in0=am[:, lo:lo + 128],
                                            in1=cmask_d[:], op=ALU.mult)
                    att.append(am)
                po_t = opsum.tile([65, K], F32, tag="po")
                for jc in range(NI):
                    lo = jc * 128
                    nc.tensor.matmul(po_t[:, lo:], vte_jc[jc][:, h2, 0:65],
                                     att[jc][:, lo:],
                                     start=(jc == 0), stop=(jc == 3))
                # evict raw o + Z strip (lane-aligned); PSUM freed here
                if h % 2 == 0:
                    nc.vector.tensor_copy(o2[qch][0:64, :], po_t[0:64, :])
                else:
                    orw = praw.tile([64, K], BF16, tag="orw")
                    raw_odd[h] = orw
                    nc.vector.tensor_copy(orw[:], po_t[0:64, :])
                nc.scalar.activation(zs64[64:65, h % 8, :], po_t[64:65, :],
                                     AF.Copy)
                if h % 8 == 7:
                    # shuffle 8 Z strips onto partitions 0..7, batch-
                    # reciprocal, bounce back to partition 0; per head,
                    # broadcast 1/Z across 64 partitions with a tiny PE
                    # matmul (lane-aligned) and divide into the raw output
                    hs = h - 7
                    ztb = zall.tile([8, K], BF16, tag="ztb")
                    nc.sync.dma_start(out=ztb[:], in_=zs64[64:65, :, :])
                    zrf = zall.tile([8, K], F32, tag="zrf")
                    nc.vector.reciprocal(zrf[:], ztb[:])
                    zrb = zall.tile([8, K], BF16, tag="zrb")
                    nc.vector.tensor_copy(zrb[:], zrf[:])
                    zrbT = zall.tile([1, 8, K], BF16, tag="zrbT")
                    nc.sync.dma_start(out=zrbT[:], in_=zrb[:])
                    for hh in range(hs, hs + 8):
                        pz = zpsum.tile([64, K], F32, tag="pz")
                        nc.tensor.matmul(pz[:], ones_row[0:1, 0:64],
                                         zrbT[0:1, hh - hs, :],
                                         start=True, stop=True)
                        if hh % 2 == 0:
                            nc.vector.tensor_tensor(
                                out=o2[hh // 2][0:64, :],
                                in0=o2[hh // 2][0:64, :], in1=pz[:],
                                op=ALU.mult)
                        else:
                            orw = raw_odd.pop(hh)
                            nc.vector.tensor_tensor(
                                out=orw[:], in0=orw[:], in1=pz[:],
                                op=ALU.mult)
                            nc.sync.dma_start(
                                out=o2[hh // 2][64:128, :], in_=orw[:])

    if KSTOP == 5:
        for hp in range(8):
            nc.gpsimd.dma_start(out=out_d[hp * 128:(hp + 1) * 128, 0:K],
                                in_=o2[hp][:])
        return

    # ---------------- stage 6: Wo (paired heads) + residual ----------------
    with tc.tile_pool(name="aopsum", bufs=2, space="PSUM") as aopsum:
        for i in range(NI):
            pao = aopsum.tile([128, C], F32, tag="pao")
            for hp in range(8):
                for nh in range(2):
                    nc.tensor.matmul(
                        pao[:, nh * 512:(nh + 1) * 512],
                        o2[hp][:, i * 128:(i + 1) * 128],
                        wo_sb[hp][:, nh * 512:(nh + 1) * 512],
                        start=(hp == 0), stop=(hp == 7))
            nc.vector.tensor_tensor(out=y1[:, i, :], in0=pao[:],
                                    in1=y0[:, i, :], op=ALU.add)

    if KSTOP == 6:
        for c in range(NI):
            nc.sync.dma_start(out=out_d[c * 128:(c + 1) * 128, :],
                              in_=y1[:, c, :])
        return
    pq_stack.close()

    # ---------------- stage 7: LN2 + transpose ----------------
    pmlp = es.enter_context(tc.tile_pool(name="pmlp", bufs=1))
    mT8 = pmlp.tile([128, NCC, K], FP8, name="mT8")
    h2big = pmlp.tile([128, NFC, K], FP8, name="h2big")
    w2_sb = []
    for fp in range(16):
        w2t = pmlp.tile([128, 2, C], FP8, name=f"w2_{fp}")
        for i in range(2):
            r = (2 * fp + i) * 128
            nc.sync.dma_start(out=w2t[:, i, :], in_=w2_d[r:r + 128, :])
        w2_sb.append(w2t)
    with tc.tile_pool(name="mbuf", bufs=2) as mbuf, \
         tc.tile_pool(name="tpsum2", bufs=4, space="PSUM") as tpsum2:
        for i in range(NI):
            mti = mbuf.tile([128, C], BF16, tag="m")
            ln_tokmajor(y1[:, i, :], mti[:])
            for cc in range(NCC):
                pt = tpsum2.tile([128, 128], BF16, tag="tp2")
                nc.tensor.transpose(pt[:], mti[:, cc * 128:(cc + 1) * 128],
                                    ident[:])
                nc.vector.tensor_copy(
                    mT8[:, cc, i * 128:(i + 1) * 128], pt[:])

    # ---------------- stage 8: W1/W3 (fp8 DoubleRow) + SwiGLU -------------
    with tc.tile_pool(name="w13p", bufs=2) as w13p, \
         tc.tile_pool(name="upsum", bufs=2, space="PSUM") as upsum, \
         tc.tile_pool(name="gpsum", bufs=2, space="PSUM") as gpsum, \
         tc.tile_pool(name="sbuf8", bufs=3) as sbuf8:
        for fg in range(4):               # groups of 8 ffn chunks
            w1g, w3g = [], []
            for jp in range(NCP):
                t1 = w13p.tile([128, 2, 1024], FP8, tag=f"w1g{jp}")
                t3 = w13p.tile([128, 2, 1024], FP8, tag=f"w3g{jp}")
                for i in range(2):
                    r = (2 * jp + i) * 128
                    nc.sync.dma_start(
                        out=t1[:, i, :],
                        in_=w1_d[r:r + 128, fg * 1024:(fg + 1) * 1024])
                    nc.sync.dma_start(
                        out=t3[:, i, :],
                        in_=w3_d[r:r + 128, fg * 1024:(fg + 1) * 1024])
                w1g.append(t1)
                w3g.append(t3)
            for fi in range(8):
                f = fg * 8 + fi
                pu = upsum.tile([128, K], F32, tag="pu")
                pg = gpsum.tile([128, K], F32, tag="pg")
                for jp in range(NCP):
                    nc.tensor.matmul(pu[:],
                                     w1g[jp][:, :, fi * 128:(fi + 1) * 128],
                                     mT8[:, 2 * jp:2 * jp + 2, :],
                                     start=(jp == 0), stop=(jp == NCP - 1),
                                     perf_mode=DR)
                for jp in range(NCP):
                    nc.tensor.matmul(pg[:],
                                     w3g[jp][:, :, fi * 128:(fi + 1) * 128],
                                     mT8[:, 2 * jp:2 * jp + 2, :],
                                     start=(jp == 0), stop=(jp == NCP - 1),
                                     perf_mode=DR)
                # u = pu * INV;  h2 (scaled x16 for fp8) = us * pg * INV*16
                sg = sbuf8.tile([128, K], BF16, tag="sg")
                nc.scalar.activation(sg[:], pu[:], AF.Sigmoid, scale=INV)
                us = sbuf8.tile([128, K], BF16, tag="us")
                nc.vector.scalar_tensor_tensor(out=us[:], in0=pu[:],
                                               scalar=INV, in1=sg[:],
                                               op0=ALU.mult, op1=ALU.mult)
                nc.vector.scalar_tensor_tensor(out=h2big[:, f, :], in0=pg[:],
                                               scalar=INV * AS, in1=us[:],
                                               op0=ALU.mult, op1=ALU.mult)

    if KSTOP == 8:
        with tc.tile_pool(name="dbg8", bufs=2) as dbg8:
            for f in range(8):
                db = dbg8.tile([128, K], F32, tag="db")
                nc.vector.tensor_copy(db[:], h2big[:, f, :])
                nc.sync.dma_start(out=out_d[f * 128:(f + 1) * 128, 0:K],
                                  in_=db[:])
        return

    # ---------------- stage 9: W2 (fp8 DoubleRow) + residual + w-scale ----
    # W2 is fully resident (prefetched at stage 7); per-chunk scatter-add
    # overlaps the remaining chunks' matmuls
    with tc.tile_pool(name="mpsum", bufs=2, space="PSUM") as mpsum, \
         tc.tile_pool(name="y2buf", bufs=2) as y2buf:
        for i in range(NI):
            pm = mpsum.tile([128, C], F32, tag="pm")
            for fp in range(16):
                for nh in range(2):
                    nc.tensor.matmul(
                        pm[:, nh * 512:(nh + 1) * 512],
                        h2big[:, 2 * fp:2 * fp + 2, i * 128:(i + 1) * 128],
                        w2_sb[fp][:, :, nh * 512:(nh + 1) * 512],
                        start=(fp == 0), stop=(fp == 15),
                        perf_mode=DR)
            y2t = y2buf.tile([128, C], F32, tag="y2")
            nc.vector.scalar_tensor_tensor(out=y2t[:], in0=pm[:],
                                           scalar=INV, in1=y1[:, i, :],
                                           op0=ALU.mult, op1=ALU.add)
            # swr reuses y0's storage (y0 is dead after stage 6)
            nc.scalar.activation(y0[:, i, :], y2t[:], AF.Copy,
                                 scale=w128[:, i:i + 1])
            nc.gpsimd.dma_scatter_add(out_ap=out_d[:, :],
                                      in_ap=y0[:, i:i + 1, :],
                                      idxs_ap=idx128[:, 8 * i:8 * i + 8],
                                      num_idxs=128, num_idxs_reg=128,
                                      elem_size=C)


_CACHE = {}


def _get_compiled():
    if "nc" in _CACHE:
        return _CACHE["nc"]
    from contextlib import ExitStack
    nc = bacc.Bacc("TRN2", target_bir_lowering=False, debug=False)
    with tile.TileContext(nc) as tc:
        with ExitStack() as es:
            build(nc, tc, es)
    nc.compile()
    _CACHE["nc"] = nc
    return nc


def _prep_host_inputs(inputs):
    x = np.asarray(inputs["x"], dtype=np.float32)          # (8, 4096, 1024)
    Wr = np.asarray(inputs["Wr"], dtype=np.float32)
    ln1_g = np.asarray(inputs["ln1_g"], dtype=np.float32)
    ln2_g = np.asarray(inputs["ln2_g"], dtype=np.float32)
    f8 = ml_dtypes.float8_e4m3fn

    def q8(a):
        return np.clip(a * WS, -240.0, 240.0).astype(f8)

    wqkv = q8(np.asarray(inputs["Wqkv"], np.float32) * ln1_g[:, None])
    wo = np.asarray(inputs["Wo"], np.float32).astype(ml_dtypes.bfloat16)
    w1 = q8(np.asarray(inputs["W1"], np.float32) * ln2_g[:, None])
    w3 = q8(np.asarray(inputs["W3"], np.float32) * ln2_g[:, None])
    w2 = q8(np.asarray(inputs["W2"], np.float32))
    shared = {
        "wr": np.ascontiguousarray(Wr[None, :]),
        "wqkv": np.ascontiguousarray(wqkv),
        "wo": np.ascontiguousarray(wo),
        "w1": np.ascontiguousarray(w1),
        "w3": np.ascontiguousarray(w3),
        "w2": np.ascontiguousarray(w2),
    }
    return [{"x": np.ascontiguousarray(x[b]), **shared} for b in range(B)]


def kernel(**inputs):
    nc = _get_compiled()
    in_maps = _prep_host_inputs(inputs)
    res = run_bass_kernel_spmd(nc, in_maps, core_ids=list(range(N_CORES)))
    _CACHE["last_results"] = res
    out = np.stack([res.results[b]["out"] for b in range(B)], axis=0)
    return out.astype(np.float32)
